# revision 2
# baseline (speedup 1.0000x reference)
import sys

sys.path.insert(0, "/opt/trn_rl_repo")

import numpy as np
from contextlib import ExitStack

# Problem constants (hardcoded per contract: kernel.py is self-contained).
B, S, D, O, M, E = 8, 2048, 768, 512, 1536, 8
T = S  # tokens per core (data-parallel over batch: 1 batch row per core)
P = 128
DT = D // P   # 6 d-tiles
MT = M // P   # 12 m-tiles
NT = T // P   # 16 token tiles per core
NCORES = 8

_CACHE = {}


def _build():
    import concourse.bass as bass
    import concourse.tile as tile
    from concourse import bacc, mybir
    from concourse.masks import make_identity

    f32 = mybir.dt.float32
    bf16 = mybir.dt.bfloat16
    AF = mybir.ActivationFunctionType
    ALU = mybir.AluOpType

    nc = bacc.Bacc("TRN2", target_bir_lowering=False, debug=False,
                   num_devices=NCORES)

    x_d = nc.dram_tensor("x", (T, D), f32, kind="ExternalInput").ap()
    wg_d = nc.dram_tensor("w_gate", (D, E), f32, kind="ExternalInput").ap()
    bi_d = nc.dram_tensor("bias_in", (E, D), f32, kind="ExternalInput").ap()
    win_d = nc.dram_tensor("W_in", (E, M, D), f32, kind="ExternalInput").ap()
    wout_d = nc.dram_tensor("W_out", (E, O, M), f32, kind="ExternalInput").ap()
    bo_d = nc.dram_tensor("b_out", (E, O), f32, kind="ExternalInput").ap()
    wsc_d = nc.dram_tensor("W_sc", (E, O, D), f32, kind="ExternalInput").ap()
    out_d = nc.dram_tensor("out", (T, O), f32, kind="ExternalOutput").ap()

    with tile.TileContext(nc) as tc, ExitStack() as ctx:
        const = ctx.enter_context(tc.tile_pool(name="const", bufs=1))
        stage = ctx.enter_context(tc.tile_pool(name="stage", bufs=2))
        wt = ctx.enter_context(tc.tile_pool(name="wt", bufs=2))
        hp = ctx.enter_context(tc.tile_pool(name="hp", bufs=1))
        comb = ctx.enter_context(tc.tile_pool(name="comb", bufs=2))
        pmm1 = ctx.enter_context(tc.tile_pool(name="pmm1", bufs=4, space="PSUM"))
        pmm2 = ctx.enter_context(tc.tile_pool(name="pmm2", bufs=2, space="PSUM"))
        ptr = ctx.enter_context(tc.tile_pool(name="ptr", bufs=2, space="PSUM"))

        ident = const.tile([P, P], bf16)
        make_identity(nc, ident)

        # ---- persistent SBUF tensors ----
        xT = const.tile([P, DT, T], bf16)        # x transposed: [d_inner, d_outer, t]
        acc = const.tile([P, NT, O], f32)        # output accumulator [t_in, t_out, o]
        g_exp = const.tile([P, NT, E], f32)      # unnormalized softmax numerators
        g_bf = const.tile([P, NT, E], bf16)
        rinv = const.tile([P, NT], f32)          # 1 / sum_e exp
        gsum = const.tile([P, NT], f32)
        gTexp = const.tile([P, NT, P], bf16)     # gates transposed [e<=8, tt, t_in]
        wgate_f = const.tile([P, DT, E], f32)
        wgate_sb = const.tile([P, DT, E], bf16)
        bias_f = const.tile([P, D], f32)         # bias_in rows on first 8 partitions
        bias_bf = const.tile([P, D], bf16)
        biasT = const.tile([P, DT, E], bf16)     # bias_in transposed [d_in, d_out, e]
        bo_f = const.tile([P, O], f32)           # b_out on first 8 partitions
        bo_sb = const.tile([P, O], bf16)

        # ---- small input loads ----
        nc.sync.dma_start(wgate_f, wg_d.rearrange("(po pi) e -> pi po e", pi=P))
        nc.vector.tensor_copy(wgate_sb, wgate_f)
        nc.sync.dma_start(bias_f[:E, :], bi_d)
        nc.vector.tensor_copy(bias_bf[:E, :], bias_f[:E, :])
        nc.sync.dma_start(bo_f[:E, :], bo_d)
        nc.vector.tensor_copy(bo_sb[:E, :], bo_f[:E, :])

        # transpose bias_in -> biasT  (blocks of [8,128] -> [128,8])
        for g in range(2):
            pt = ptr.tile([P, 4, P], bf16, tag="tr")
            n = 4 if g == 0 else 2
            for i in range(n):
                dt_ = g * 4 + i
                nc.tensor.transpose(pt[:, i, :E], bias_bf[:E, dt_ * P:(dt_ + 1) * P],
                                    ident[:E, :E])
            nc.vector.tensor_copy(biasT[:, g * 4:g * 4 + n, :], pt[:, :n, :E])

        # ---- x load, cast, transpose + gating matmul ----
        for tt in range(NT):
            xs = stage.tile([P, D], f32, tag="sf32", bufs=3)
            nc.scalar.dma_start(xs, x_d[tt * P:(tt + 1) * P, :])
            xb = stage.tile([P, D], bf16, tag="wbf", bufs=12)
            nc.gpsimd.tensor_copy(xb, xs)
            for g in range(2):
                pt = ptr.tile([P, 4, P], bf16, tag="tr")
                n = 4 if g == 0 else 2
                for i in range(n):
                    dt_ = g * 4 + i
                    nc.tensor.transpose(pt[:, i, :], xb[:, dt_ * P:(dt_ + 1) * P],
                                        ident)
                nc.vector.tensor_copy(xT[:, g * 4:g * 4 + n, tt * P:(tt + 1) * P],
                                      pt[:, :n, :])
            # gating logits for this token tile: [128, 8]
            pg = pmm2.tile([P, O], f32, tag="mm2")
            for dt_ in range(DT):
                nc.tensor.matmul(pg[:, :E], xT[:, dt_, tt * P:(tt + 1) * P],
                                 wgate_sb[:, dt_, :],
                                 start=(dt_ == 0), stop=(dt_ == DT - 1))
            nc.scalar.activation(g_exp[:, tt, :], pg[:, :E], AF.Exp)

        nc.vector.tensor_reduce(gsum, g_exp, axis=mybir.AxisListType.X, op=ALU.add)
        nc.vector.reciprocal(rinv, gsum)
        nc.gpsimd.tensor_copy(g_bf, g_exp)

        # transpose gates ([128,8] blocks -> [8,128]) for the b_out init matmul
        for g in range(4):
            pt = ptr.tile([P, 4, P], bf16, tag="tr")
            for i in range(4):
                tt = g * 4 + i
                nc.tensor.transpose(pt[:E, i, :], g_bf[:, tt, :], ident)
            nc.vector.tensor_copy(gTexp[:E, g * 4:(g + 1) * 4, :], pt[:E, :4, :])

        # acc init: acc[t, o] = (g_exp[t, :] @ b_out) * rinv[t]
        for tt in range(NT):
            pb = pmm2.tile([P, O], f32, tag="mm2")
            nc.tensor.matmul(pb, gTexp[:E, tt, :], bo_sb[:E, :])
            nc.vector.tensor_scalar_mul(acc[:, tt, :], pb,
                                        scalar1=rinv[:, tt:tt + 1])

        # ---- expert weight load helpers ----
        def load_expert(e):
            """DMA + cast; returns staged bf16 chunks to transpose on PE later."""
            chunks = []  # (bf16_stage, kind, row)
            for r in range(MT):  # W_in rows: 12 chunks of [128, 768]
                ws = stage.tile([P, D], f32, tag="sf32", bufs=3)
                nc.sync.dma_start(ws, win_d[e, r * P:(r + 1) * P, :])
                wb = stage.tile([P, D], bf16, tag="wbf", bufs=12)
                nc.vector.tensor_copy(wb, ws)
                chunks.append((wb, "win", r))
            for r in range(O // P):  # W_out rows: 4 x 2 halves of [128, 768]
                for h in range(2):
                    ws = stage.tile([P, D], f32, tag="sf32", bufs=3)
                    nc.scalar.dma_start(
                        ws, wout_d[e, r * P:(r + 1) * P, h * D:(h + 1) * D])
                    wb = stage.tile([P, D], bf16, tag="wbf", bufs=12)
                    nc.vector.tensor_copy(wb, ws)
                    chunks.append((wb, "wout", r * 2 + h))
            for r in range(O // P):  # W_sc rows: 4 chunks of [128, 768]
                ws = stage.tile([P, D], f32, tag="sf32", bufs=3)
                nc.sync.dma_start(ws, wsc_d[e, r * P:(r + 1) * P, :])
                wb = stage.tile([P, D], bf16, tag="wbf", bufs=12)
                nc.vector.tensor_copy(wb, ws)
                chunks.append((wb, "wsc", r))
            winT = wt.tile([P, DT, M], bf16, tag="winT")
            woutT = wt.tile([P, MT, O], bf16, tag="woutT")
            wscT = wt.tile([P, DT, O], bf16, tag="wscT")
            return chunks, winT, woutT, wscT

        def transpose_expert(chunks, winT, woutT, wscT):
            for wb, kind, r in chunks:
                for g in range(2):
                    pt = ptr.tile([P, 4, P], bf16, tag="tr")
                    n = 4 if g == 0 else 2
                    for i in range(n):
                        blk = g * 4 + i
                        nc.tensor.transpose(pt[:, i, :], wb[:, blk * P:(blk + 1) * P],
                                            ident)
                    if kind == "win":
                        dst = winT[:, g * 4:g * 4 + n, r * P:(r + 1) * P]
                    elif kind == "wout":
                        # chunk r covers o-rows (r//2)*128, m-cols (r%2)*768
                        mt0 = (r % 2) * DT + g * 4
                        o0 = (r // 2) * P
                        dst = woutT[:, mt0:mt0 + n, o0:o0 + P]
                    else:
                        dst = wscT[:, g * 4:g * 4 + n, r * P:(r + 1) * P]
                    nc.vector.tensor_copy(dst, pt[:, :n, :])

        def compute_c(e, winT, neg_c):
            # c[e, m] = sum_d bias_in[e, d] * W_in[e, m, d]; store -c per m-tile
            for mt in range(MT):
                pc = pmm2.tile([P, O], f32, tag="mm2")
                for dt_ in range(DT):
                    nc.tensor.matmul(pc[:, 0:1], winT[:, dt_, mt * P:(mt + 1) * P],
                                     biasT[:, dt_, e:e + 1],
                                     start=(dt_ == 0), stop=(dt_ == DT - 1))
                nc.vector.tensor_scalar_mul(neg_c[:, mt:mt + 1], pc[:, 0:1],
                                            scalar1=-1.0)

        hT = hp.tile([P, MT, T // 2], bf16)

        # preload expert 0
        chunks0, winT, woutT, wscT = load_expert(0)
        transpose_expert(chunks0, winT, woutT, wscT)
        neg_c = wt.tile([P, MT], f32, tag="negc")
        compute_c(0, winT, neg_c)

        for e in range(E):
            nxt = None
            if e + 1 < E:
                nxt_chunks, nwinT, nwoutT, nwscT = load_expert(e + 1)
                nxt = (nxt_chunks, nwinT, nwoutT, nwscT)

            for th in range(2):
                t0 = th * (T // 2)
                # mm1: hT[m, t] = gelu(W_in[e] @ x^T - c)
                for mt in range(MT):
                    for tq in range(2):
                        ph = pmm1.tile([P, O], f32, tag="mm1")
                        for dt_ in range(DT):
                            nc.tensor.matmul(
                                ph, winT[:, dt_, mt * P:(mt + 1) * P],
                                xT[:, dt_, t0 + tq * O:t0 + (tq + 1) * O],
                                start=(dt_ == 0), stop=(dt_ == DT - 1))
                        nc.scalar.activation(hT[:, mt, tq * O:(tq + 1) * O], ph,
                                             AF.Gelu, bias=neg_c[:, mt:mt + 1],
                                             scale=1.0)

                # after mm1 of the first half, transpose next expert's weights
                if th == 0 and nxt is not None:
                    transpose_expert(*nxt)
                    nneg_c = wt.tile([P, MT], f32, tag="negc")
                    compute_c(e + 1, nxt[1], nneg_c)

                # mm2 + mm3 + gate-weighted accumulate
                for t8 in range(8):
                    tg = th * 8 + t8
                    po = pmm2.tile([P, O], f32, tag="mm2")
                    for mt in range(MT):
                        nc.tensor.matmul(po, hT[:, mt, t8 * P:(t8 + 1) * P],
                                         woutT[:, mt, :],
                                         start=(mt == 0), stop=False)
                    for dt_ in range(DT):
                        nc.tensor.matmul(po, xT[:, dt_, tg * P:(tg + 1) * P],
                                         wscT[:, dt_, :],
                                         start=False, stop=(dt_ == DT - 1))
                    tmp = comb.tile([P, O], f32, tag="tmp")
                    nc.vector.tensor_scalar(out=tmp, in0=po,
                                            scalar1=g_exp[:, tg, e:e + 1],
                                            scalar2=rinv[:, tg:tg + 1],
                                            op0=ALU.mult, op1=ALU.mult)
                    nc.gpsimd.tensor_add(acc[:, tg, :], acc[:, tg, :], tmp)
                    if e == E - 1:
                        nc.sync.dma_start(out_d[tg * P:(tg + 1) * P, :],
                                          acc[:, tg, :])

            if nxt is not None:
                winT, woutT, wscT = nxt[1], nxt[2], nxt[3]
                neg_c = nneg_c

    nc.compile()
    return nc


def _get_nc():
    if "nc" not in _CACHE:
        _CACHE["nc"] = _build()
    return _CACHE["nc"]


def kernel(x, w_gate, bias_in, W_in, W_out, b_out, W_sc):
    from concourse.bass_utils import run_bass_kernel_spmd

    nc = _get_nc()
    x = np.ascontiguousarray(np.asarray(x, dtype=np.float32))
    shared = {
        "w_gate": np.ascontiguousarray(np.asarray(w_gate, dtype=np.float32)),
        "bias_in": np.ascontiguousarray(np.asarray(bias_in, dtype=np.float32)),
        "W_in": np.ascontiguousarray(np.asarray(W_in, dtype=np.float32)),
        "W_out": np.ascontiguousarray(np.asarray(W_out, dtype=np.float32)),
        "b_out": np.ascontiguousarray(np.asarray(b_out, dtype=np.float32)),
        "W_sc": np.ascontiguousarray(np.asarray(W_sc, dtype=np.float32)),
    }
    in_maps = [{"x": x[i], **shared} for i in range(NCORES)]
    res = run_bass_kernel_spmd(nc, in_maps, core_ids=list(range(NCORES)))
    out = np.stack([res.results[i]["out"] for i in range(NCORES)], axis=0)
    return out.astype(np.float32)
